# revision 22
# baseline (speedup 1.0000x reference)
"""Multi-head dot-product attention (B=2, S=2048, F=1024, H=16, DH=64, O=1024)
as a Bass/Tile kernel on 8 Trainium2 NeuronCores.

Sharding: data-parallel over B (2) x tensor-parallel over H (4 groups of 4
heads) = 8 cores. Each core computes q/k/v projections for its 4 heads,
softmax attention, and a partial output projection; the host sums the 4
partial outputs per batch element and adds the bias.

Device layouts (per core):
  xqT, xkvT  [F, S]  fp16   host-pre-transposed activations
  wq, wk, wv [F, 4*DH] fp16 weight shards (wq pre-scaled by 1/sqrt(DH))
  wo         [4*DH, O] fp16
  out        [S, O]  fp32   partial output

Attention works in transposed-score space: sT[k, q] = KT_slice.T @ QT (two
heads packed into PE row-groups 0-63 / 64-127), one exp on ACT covers both
heads, then y'T = V'.T @ PT where V' carries a ones column so row 64 of y'T
accumulates the softmax denominator (scores are O(1), so max-subtraction is
unnecessary). The denominator row is broadcast across partitions with a K=1
fp32r ones-matmul, reciprocated on DVE, and multiplied into fp16 yT tiles
used as lhsT of the output projection.

The emission is software-pipelined: scores run one k-tile ahead of the
exp-dependent y matmuls, the next block's first scores are issued before the
current block's normalization, and normalization + output-projection work is
spread through the following block's kt loop in sub-microsecond units so the
ACT engine (the throughput floor) never stalls.
"""

import numpy as np

import concourse.bass as bass
import concourse.mybir as mybir
import concourse.tile as tile
from concourse import bacc
from concourse.bass_utils import run_bass_kernel_spmd

F32 = mybir.dt.float32
F32R = mybir.dt.float32r
F16 = mybir.dt.float16
AF = mybir.ActivationFunctionType

B, S, F, H, DH, O = 2, 2048, 1024, 16, 64, 1024
NCORES = 8
HPC = 4  # heads per core
CH = 512  # q-chunk width
P = 128


def build_program(s=S, f=F, o=O, hpc=HPC):
    npair = hpc // 2
    nch = s // CH  # q chunks
    nkt = s // P  # k tiles
    nf = f // P  # contraction tiles for projections
    hd = hpc * DH  # stacked head dims per core (256)

    nc = bacc.Bacc("TRN2", target_bir_lowering=False, debug=False, num_devices=NCORES)

    xqT = nc.dram_tensor("xqT", [f, s], F16, kind="ExternalInput")
    xkvT = nc.dram_tensor("xkvT", [f, s], F16, kind="ExternalInput")
    wq = nc.dram_tensor("wq", [f, hd], F16, kind="ExternalInput")
    wk = nc.dram_tensor("wk", [f, hd], F16, kind="ExternalInput")
    wv = nc.dram_tensor("wv", [f, hd], F16, kind="ExternalInput")
    wo = nc.dram_tensor("wo", [hd, o], F16, kind="ExternalInput")
    out = nc.dram_tensor("out", [s, o], F32, kind="ExternalOutput")

    xqT_t = xqT.ap().rearrange("(t p) n -> p t n", p=P)  # [128, nf, s]
    xkvT_t = xkvT.ap().rearrange("(t p) n -> p t n", p=P)
    wq_t = wq.ap().rearrange("(t p) n -> p t n", p=P)  # [128, nf, hd]
    wk_t = wk.ap().rearrange("(t p) n -> p t n", p=P)
    wv_t = wv.ap().rearrange("(t p) n -> p t n", p=P)
    wo_t = wo.ap().rearrange("(t p) n -> p t n", p=P)  # [128, hd//128, o]

    with tile.TileContext(nc) as tc:
        with (
            tc.tile_pool(name="weights", bufs=1) as wpool,
            tc.tile_pool(name="xin", bufs=2) as xpool,
            tc.tile_pool(name="qkv", bufs=1) as qkvpool,
            tc.tile_pool(name="pt", bufs=1) as ptpool,
            tc.tile_pool(name="norm", bufs=4) as npool,
            tc.tile_pool(name="outsb", bufs=2) as opool,
        ):
            # ---- weights + constants -------------------------------------
            # per-f-tile wq/wk tiles so the first matmuls only wait on a
            # small DMA; weight DMAs ride the ACT HWDGE ring so they don't
            # head-of-line block the x stream on the SP ring
            wq_f = [wpool.tile([P, hd], F16, tag=f"wq{t}", name=f"wq{t}") for t in range(nf)]
            wk_f = [wpool.tile([P, hd], F16, tag=f"wk{t}", name=f"wk{t}") for t in range(nf)]
            wv_sb = wpool.tile([P, nf, hd], F16, tag="wv")
            wo_sb = wpool.tile([P, hd // P, o], F16, tag="wo")
            for ft in range(nf):
                nc.scalar.dma_start(wq_f[ft][:], wq_t[:, ft])
                nc.scalar.dma_start(wk_f[ft][:], wk_t[:, ft])
            nc.scalar.dma_start(wv_sb[:], wv_t)
            nc.scalar.dma_start(wo_sb[:], wo_t)
            # memset can't write fp16/fp32r; memset fp32 scratch, cast-copy
            ones_f32 = wpool.tile([P, P], F32, tag="ones_f32")
            nc.vector.memset(ones_f32[:], 1.0)
            ones_sb = wpool.tile([1, P], F32R, tag="ones")
            nc.vector.tensor_copy(ones_sb[:], ones_f32[0:1, :])

            # ---- storage -------------------------------------------------
            QT = [
                [qkvpool.tile([P, CH], F16, tag=f"QT{p_}_{c}", name=f"QT{p_}_{c}") for c in range(nch)]
                for p_ in range(npair)
            ]
            KT = [
                [qkvpool.tile([P, CH], F16, tag=f"KT{p_}_{c}", name=f"KT{p_}_{c}") for c in range(nch)]
                for p_ in range(npair)
            ]
            # V': per k-tile [128, hpc, DH+1]; last column is ones
            V = [qkvpool.tile([P, hpc, DH + 1], F16, tag=f"V{kt}", name=f"V{kt}") for kt in range(nkt)]
            YT = [
                [qkvpool.tile([P, CH], F16, tag=f"YT{p_}_{c}", name=f"YT{p_}_{c}") for c in range(nch)]
                for p_ in range(npair)
            ]
            for kt in range(nkt):
                nc.vector.tensor_copy(V[kt][:, :, DH], ones_f32[:, 0:hpc])

            # ps_s (scores / broadcast / out-proj PSUM) lives for the whole
            # kernel: 4 banks. Projection-phase pools add 4 more (within the
            # 8-bank budget); after they close, the psY pool takes 4.
            with tc.tile_pool(name="ps_att", bufs=2, space="PSUM") as ps_att:
                blocks = [(c, p_) for c in range(nch) for p_ in range(npair)]

                def emit_scores(p_, c, kt):
                    ps_s = ps_att.tile([P, 2 * CH], F32, tag="ps_s", name="ps_s")
                    nc.tensor.matmul(
                        ps_s[:, 0:CH],
                        KT[p_][kt // 4][0:DH, (kt % 4) * P : (kt % 4 + 1) * P],
                        QT[p_][c][0:DH, :],
                        tile_position=(0, 0),
                    )
                    nc.tensor.matmul(
                        ps_s[:, CH : 2 * CH],
                        KT[p_][kt // 4][DH : 2 * DH, (kt % 4) * P : (kt % 4 + 1) * P],
                        QT[p_][c][DH : 2 * DH, :],
                        tile_position=(DH, 0),
                    )
                    return ps_s

                # saved exp(scores) tiles, written one block ahead of their
                # y-matmuls; (block parity, kt) keys the SBUF slot
                PT = {}

                def emit_score_exp(bi, kt):
                    c, p_ = blocks[bi]
                    ps_s = emit_scores(p_, c, kt)
                    pt = ptpool.tile(
                        [P, 2 * CH], F16, tag=f"pt{bi % 2}_{kt}", name=f"pt{bi % 2}_{kt}"
                    )
                    nc.scalar.activation(pt[:], ps_s[:], AF.Exp)
                    PT[(bi, kt)] = pt

                # ---- projections (+ block0 scores/exp hidden under them) --
                with (
                    tc.tile_pool(name="ps_projqk", bufs=1, space="PSUM") as ps_projqk,
                    tc.tile_pool(name="ps_projv", bufs=2, space="PSUM") as ps_projv,
                ):
                    # PE warm-up: dummy matmuls with no DMA dependency keep
                    # the PE busy through the HAM activity window so the real
                    # projection matmuls start at 2.4GHz instead of 1.2GHz
                    for wu in range(40):
                        ps_wu = ps_projv.tile([P, P], F32, tag="psV", name="ps_wu")
                        nc.tensor.matmul(ps_wu[:], ones_sb[:], ones_sb[:])
                    for c in range(nch):
                        # one 1MB DMA per chunk per stream (SP-side dispatch
                        # for [128,512] slices costs ~0.7us each)
                        xq_t = xpool.tile([P, nf, CH], F16, tag="xq", name="xq_t")
                        nc.sync.dma_start(xq_t[:], xqT_t[:, :, c * CH : (c + 1) * CH])
                        xkv_t = xpool.tile([P, nf, CH], F16, tag="xkv", name="xkv_t")
                        nc.sync.dma_start(xkv_t[:], xkvT_t[:, :, c * CH : (c + 1) * CH])
                        # Q pass (K pass reuses the same PSUM tags)
                        psQ = [ps_projqk.tile([P, CH], F32, tag=f"psQK{m}", name="psQ") for m in range(npair)]
                        for ft in range(nf):
                            for m in range(npair):
                                nc.tensor.matmul(
                                    psQ[m][:],
                                    wq_f[ft][:, m * P : (m + 1) * P],
                                    xq_t[:, ft],
                                    start=(ft == 0),
                                    stop=(ft == nf - 1),
                                )
                        for m in range(npair):
                            nc.vector.tensor_copy(QT[m][c][:], psQ[m][:])
                        # K pass
                        psK = [ps_projqk.tile([P, CH], F32, tag=f"psQK{m}", name="psK") for m in range(npair)]
                        for ft in range(nf):
                            for m in range(npair):
                                nc.tensor.matmul(
                                    psK[m][:],
                                    wk_f[ft][:, m * P : (m + 1) * P],
                                    xkv_t[:, ft],
                                    start=(ft == 0),
                                    stop=(ft == nf - 1),
                                )
                        for m in range(npair):
                            nc.vector.tensor_copy(KT[m][c][:], psK[m][:])
                        # V pass (xkv chunk tile as lhsT); one PSUM
                        # accumulation group per bank, so st is outer
                        for st in range(4):
                            psV = ps_projv.tile([P, CH], F32, tag="psV", name="psV")
                            for ft in range(nf):
                                nc.tensor.matmul(
                                    psV[:, 0:hd],
                                    xkv_t[:, ft, st * P : (st + 1) * P],
                                    wv_sb[:, ft, :],
                                    start=(ft == 0),
                                    stop=(ft == nf - 1),
                                )
                            kt = c * 4 + st
                            nc.vector.tensor_copy(
                                V[kt][:, :, 0:DH],
                                psV[:, 0:hd].rearrange("p (h d) -> p h d", h=hpc),
                            )
                        # blocks 0/1's scores/exp for the k-tiles this
                        # projection chunk just enabled run on the
                        # otherwise-idle ACT engine
                        for b0 in (0, 1):
                            for kt in range(4 * c, 4 * c + 4):
                                emit_score_exp(b0, kt)

                # deferred work queue: sub-microsecond PE units injected into
                # later kt iterations so the ACT engine stays saturated
                pending = []

                def queue_normalize(p_, c, psY):
                    def emit(h01, psY=psY):
                        den_r = npool.tile([1, CH], F32R, tag="den", name="den_r")
                        nc.vector.tensor_copy(den_r[:], psY[h01][DH : DH + 1, :])
                        ps_bc = ps_att.tile([DH, CH], F32, tag="ps_s", name="ps_bc")
                        nc.tensor.matmul(ps_bc[:], ones_sb[0:1, 0:DH], den_r[:])
                        inv_sb = npool.tile([DH, CH], F32, tag="inv", name="inv_sb")
                        nc.vector.reciprocal_approx_fast(out=inv_sb[:], in_=ps_bc[:])
                        nc.vector.tensor_tensor(
                            YT[p_][c][h01 * DH : (h01 + 1) * DH, :],
                            psY[h01][0:DH, :],
                            inv_sb[:],
                            mybir.AluOpType.mult,
                        )

                    pending.append(lambda: emit(0))
                    pending.append(lambda: emit(1))

                def queue_outproj(c):
                    for st in range(4):
                        qt = c * 4 + st
                        carrier = {}

                        def emit_half(j, st=st, c=c, carrier=carrier):
                            if j == 0:
                                carrier["out_sb"] = opool.tile([P, o], F32, tag="out_sb", name="out_sb")
                            ps_o = ps_att.tile([P, CH], F32, tag="ps_s", name="ps_o")
                            for m in range(hd // P):
                                nc.tensor.matmul(
                                    ps_o[:],
                                    YT[m][c][:, st * P : (st + 1) * P],
                                    wo_sb[:, m, j * CH : (j + 1) * CH],
                                    start=(m == 0),
                                    stop=(m == hd // P - 1),
                                )
                            nc.vector.tensor_copy(
                                carrier["out_sb"][:, j * CH : (j + 1) * CH], ps_o[:]
                            )

                        def emit_dma(qt=qt, carrier=carrier):
                            nc.sync.dma_start(
                                out.ap()[qt * P : (qt + 1) * P, :], carrier["out_sb"][:]
                            )

                        pending.append(lambda f_=emit_half: f_(0))
                        pending.append(lambda f_=emit_half: f_(1))
                        pending.append(emit_dma)

                # block-level pipeline: during block bi's y-phase, block
                # bi+1's scores/exp stream on ACT (block0's ran under the
                # projections), so y-matmuls never wait on in-flight exps
                with tc.tile_pool(name="ps_y", bufs=2, space="PSUM") as ps_ypool:
                    for bi, (c, p_) in enumerate(blocks):
                        hA, hB = 2 * p_, 2 * p_ + 1
                        psY = [
                            ps_ypool.tile([DH + 1, CH], F32, tag=f"psY{h}", name=f"psY{h}")
                            for h in (0, 1)
                        ]
                        for kt in range(nkt):
                            # y first: frees the same-parity pt slot that
                            # block bi+2's exp will overwrite
                            pt = PT.pop((bi, kt))
                            nc.tensor.matmul(
                                psY[0][:],
                                V[kt][:, hA, :],
                                pt[:, 0:CH],
                                start=(kt == 0),
                                stop=(kt == nkt - 1),
                            )
                            nc.tensor.matmul(
                                psY[1][:],
                                V[kt][:, hB, :],
                                pt[:, CH : 2 * CH],
                                start=(kt == 0),
                                stop=(kt == nkt - 1),
                            )
                            if pending and (
                                kt % 2 == 1
                                or len(pending) > 6
                                or bi >= len(blocks) - 2
                            ):
                                pending.pop(0)()
                            if bi + 2 < len(blocks):
                                emit_score_exp(bi + 2, kt)
                        queue_normalize(p_, c, psY)
                        if p_ == npair - 1:
                            queue_outproj(c)
                    while pending:
                        pending.pop(0)()

    nc.compile()
    return nc


def make_in_maps(inputs_q, inputs_kv, wq, wk, wv, wo):
    """Shard full inputs into 8 per-core input dicts (host-side)."""
    in_maps = []
    scale = 1.0 / np.sqrt(DH)
    for core in range(NCORES):
        b = core // (NCORES // B)
        hg = core % (NCORES // B)
        hs = slice(hg * HPC, (hg + 1) * HPC)
        in_maps.append(
            {
                "xqT": np.ascontiguousarray(inputs_q[b].T).astype(np.float16),
                "xkvT": np.ascontiguousarray(inputs_kv[b].T).astype(np.float16),
                "wq": np.ascontiguousarray(
                    (wq[:, hs, :] * scale).reshape(F, HPC * DH)
                ).astype(np.float16),
                "wk": np.ascontiguousarray(wk[:, hs, :].reshape(F, HPC * DH)).astype(
                    np.float16
                ),
                "wv": np.ascontiguousarray(wv[:, hs, :].reshape(F, HPC * DH)).astype(
                    np.float16
                ),
                "wo": np.ascontiguousarray(wo[hs].reshape(HPC * DH, O)).astype(
                    np.float16
                ),
            }
        )
    return in_maps


_CACHE = {}


def _get_program():
    if "nc" not in _CACHE:
        _CACHE["nc"] = build_program()
    return _CACHE["nc"]


def run_sharded(inputs_q, inputs_kv, wq, wk, wv, wo, bo, **spmd_kwargs):
    """Build in_maps, run on 8 cores, reduce partials. Returns (out, results)."""
    nc = _get_program()
    in_maps = make_in_maps(inputs_q, inputs_kv, wq, wk, wv, wo)
    res = run_bass_kernel_spmd(nc, in_maps, core_ids=list(range(NCORES)), **spmd_kwargs)
    gpb = NCORES // B  # head-group cores per batch element
    out = np.zeros((B, S, O), dtype=np.float32)
    for core in range(NCORES):
        out[core // gpb] += res.results[core]["out"]
    out += np.asarray(bo, dtype=np.float32)
    return out, res


def kernel(inputs_q, inputs_kv, wq, wk, wv, wo, bo):
    out, _ = run_sharded(
        np.asarray(inputs_q),
        np.asarray(inputs_kv),
        np.asarray(wq),
        np.asarray(wk),
        np.asarray(wv),
        np.asarray(wo),
        np.asarray(bo),
    )
    return out


# revision 23
# speedup vs baseline: 1.0409x; 1.0409x over previous
"""Multi-head dot-product attention (B=2, S=2048, F=1024, H=16, DH=64, O=1024)
as a Bass/Tile kernel on 8 Trainium2 NeuronCores.

Sharding: data-parallel over B (2) x tensor-parallel over H (4 groups of 4
heads) = 8 cores. Each core computes q/k/v projections for its 4 heads,
softmax attention, and a partial output projection; the host sums the 4
partial outputs per batch element and adds the bias.

Device layouts (per core):
  xqT, xkvT  [F, S]  fp16   host-pre-transposed activations
  wq, wk, wv [F, 4*DH] fp16 weight shards (wq pre-scaled by 1/sqrt(DH))
  wo         [4*DH, O] fp16
  out        [S, O]  fp32   partial output

Attention works in transposed-score space: sT[k, q] = KT_slice.T @ QT (two
heads packed into PE row-groups 0-63 / 64-127), one exp on ACT covers both
heads, then y'T = V'.T @ PT where V' carries a ones column so row 64 of y'T
accumulates the softmax denominator (scores are O(1), so max-subtraction is
unnecessary). The denominator row is broadcast across partitions with a K=1
fp32r ones-matmul, reciprocated on DVE, and multiplied into fp16 yT tiles
used as lhsT of the output projection.

The emission is software-pipelined: scores run one k-tile ahead of the
exp-dependent y matmuls, the next block's first scores are issued before the
current block's normalization, and normalization + output-projection work is
spread through the following block's kt loop in sub-microsecond units so the
ACT engine (the throughput floor) never stalls.
"""

import numpy as np

import concourse.bass as bass
import concourse.mybir as mybir
import concourse.tile as tile
from concourse import bacc
from concourse.bass_utils import run_bass_kernel_spmd

F32 = mybir.dt.float32
F32R = mybir.dt.float32r
F16 = mybir.dt.float16
AF = mybir.ActivationFunctionType

B, S, F, H, DH, O = 2, 2048, 1024, 16, 64, 1024
NCORES = 8
HPC = 4  # heads per core
CH = 512  # q-chunk width
P = 128


def build_program(s=S, f=F, o=O, hpc=HPC):
    npair = hpc // 2
    nch = s // CH  # q chunks
    nkt = s // P  # k tiles
    nf = f // P  # contraction tiles for projections
    hd = hpc * DH  # stacked head dims per core (256)

    nc = bacc.Bacc("TRN2", target_bir_lowering=False, debug=False, num_devices=NCORES)

    xqT = nc.dram_tensor("xqT", [f, s], F16, kind="ExternalInput")
    xkvT = nc.dram_tensor("xkvT", [f, s], F16, kind="ExternalInput")
    wq = nc.dram_tensor("wq", [f, hd], F16, kind="ExternalInput")
    wk = nc.dram_tensor("wk", [f, hd], F16, kind="ExternalInput")
    wv = nc.dram_tensor("wv", [f, hd], F16, kind="ExternalInput")
    wo = nc.dram_tensor("wo", [hd, o], F16, kind="ExternalInput")
    out = nc.dram_tensor("out", [s, o], F32, kind="ExternalOutput")

    xqT_t = xqT.ap().rearrange("(t p) n -> p t n", p=P)  # [128, nf, s]
    xkvT_t = xkvT.ap().rearrange("(t p) n -> p t n", p=P)
    wq_t = wq.ap().rearrange("(t p) n -> p t n", p=P)  # [128, nf, hd]
    wk_t = wk.ap().rearrange("(t p) n -> p t n", p=P)
    wv_t = wv.ap().rearrange("(t p) n -> p t n", p=P)
    wo_t = wo.ap().rearrange("(t p) n -> p t n", p=P)  # [128, hd//128, o]

    with tile.TileContext(nc) as tc:
        with (
            tc.tile_pool(name="weights", bufs=1) as wpool,
            tc.tile_pool(name="xin", bufs=2) as xpool,
            tc.tile_pool(name="qkv", bufs=1) as qkvpool,
            tc.tile_pool(name="pt", bufs=1) as ptpool,
            tc.tile_pool(name="norm", bufs=4) as npool,
            tc.tile_pool(name="outsb", bufs=2) as opool,
        ):
            # ---- weights + constants -------------------------------------
            # per-f-tile wq/wk tiles so the first matmuls only wait on a
            # small DMA; weight DMAs ride the ACT HWDGE ring so they don't
            # head-of-line block the x stream on the SP ring
            wq_f = [wpool.tile([P, hd], F16, tag=f"wq{t}", name=f"wq{t}") for t in range(nf)]
            wk_f = [wpool.tile([P, hd], F16, tag=f"wk{t}", name=f"wk{t}") for t in range(nf)]
            wv_sb = wpool.tile([P, nf, hd], F16, tag="wv")
            wo_sb = wpool.tile([P, hd // P, o], F16, tag="wo")
            for ft in range(nf):
                nc.scalar.dma_start(wq_f[ft][:], wq_t[:, ft])
                nc.scalar.dma_start(wk_f[ft][:], wk_t[:, ft])
            nc.scalar.dma_start(wv_sb[:], wv_t)
            nc.scalar.dma_start(wo_sb[:], wo_t)
            # memset can't write fp16/fp32r; memset fp32 scratch, cast-copy
            ones_f32 = wpool.tile([P, P], F32, tag="ones_f32")
            nc.vector.memset(ones_f32[:], 1.0)
            ones_sb = wpool.tile([1, P], F32R, tag="ones")
            nc.vector.tensor_copy(ones_sb[:], ones_f32[0:1, :])

            # ---- storage -------------------------------------------------
            QT = [
                [qkvpool.tile([P, CH], F16, tag=f"QT{p_}_{c}", name=f"QT{p_}_{c}") for c in range(nch)]
                for p_ in range(npair)
            ]
            KT = [
                [qkvpool.tile([P, CH], F16, tag=f"KT{p_}_{c}", name=f"KT{p_}_{c}") for c in range(nch)]
                for p_ in range(npair)
            ]
            # V': per k-tile [128, hpc, DH+1]; last column is ones
            V = [qkvpool.tile([P, hpc, DH + 1], F16, tag=f"V{kt}", name=f"V{kt}") for kt in range(nkt)]
            YT = [
                [qkvpool.tile([P, CH], F16, tag=f"YT{p_}_{c}", name=f"YT{p_}_{c}") for c in range(nch)]
                for p_ in range(npair)
            ]
            for kt in range(nkt):
                nc.vector.tensor_copy(V[kt][:, :, DH], ones_f32[:, 0:hpc])

            # ps_s (scores / broadcast / out-proj PSUM) lives for the whole
            # kernel: 4 banks. Projection-phase pools add 4 more (within the
            # 8-bank budget); after they close, the psY pool takes 4.
            with tc.tile_pool(name="ps_att", bufs=2, space="PSUM") as ps_att:
                blocks = [(c, p_) for c in range(nch) for p_ in range(npair)]

                def emit_scores(p_, c, kt):
                    ps_s = ps_att.tile([P, 2 * CH], F32, tag="ps_s", name="ps_s")
                    nc.tensor.matmul(
                        ps_s[:, 0:CH],
                        KT[p_][kt // 4][0:DH, (kt % 4) * P : (kt % 4 + 1) * P],
                        QT[p_][c][0:DH, :],
                        tile_position=(0, 0),
                    )
                    nc.tensor.matmul(
                        ps_s[:, CH : 2 * CH],
                        KT[p_][kt // 4][DH : 2 * DH, (kt % 4) * P : (kt % 4 + 1) * P],
                        QT[p_][c][DH : 2 * DH, :],
                        tile_position=(DH, 0),
                    )
                    return ps_s

                # saved exp(scores) tiles, written one block ahead of their
                # y-matmuls; (block parity, kt) keys the SBUF slot
                PT = {}

                def emit_score_exp(bi, kt):
                    c, p_ = blocks[bi]
                    ps_s = emit_scores(p_, c, kt)
                    pt = ptpool.tile(
                        [P, 2 * CH], F16, tag=f"pt{bi % 2}_{kt}", name=f"pt{bi % 2}_{kt}"
                    )
                    nc.scalar.activation(pt[:], ps_s[:], AF.Exp)
                    PT[(bi, kt)] = pt

                # ---- projections (+ block0 scores/exp hidden under them) --
                with (
                    tc.tile_pool(name="ps_projqk", bufs=1, space="PSUM") as ps_projqk,
                    tc.tile_pool(name="ps_projv", bufs=2, space="PSUM") as ps_projv,
                ):
                    # PE warm-up: dummy matmuls with no DMA dependency keep
                    # the PE busy through the HAM activity window so the real
                    # projection matmuls start at 2.4GHz instead of 1.2GHz
                    for wu in range(40):
                        ps_wu = ps_projv.tile([P, P], F32, tag="psV", name="ps_wu")
                        nc.tensor.matmul(ps_wu[:], ones_sb[:], ones_sb[:])
                    for c in range(nch):
                        # one 1MB DMA per chunk per stream (SP-side dispatch
                        # for [128,512] slices costs ~0.7us each)
                        xq_t = xpool.tile([P, nf, CH], F16, tag="xq", name="xq_t")
                        nc.sync.dma_start(xq_t[:], xqT_t[:, :, c * CH : (c + 1) * CH])
                        xkv_t = xpool.tile([P, nf, CH], F16, tag="xkv", name="xkv_t")
                        nc.sync.dma_start(xkv_t[:], xkvT_t[:, :, c * CH : (c + 1) * CH])
                        # Q pass (K pass reuses the same PSUM tags)
                        psQ = [ps_projqk.tile([P, CH], F32, tag=f"psQK{m}", name="psQ") for m in range(npair)]
                        for ft in range(nf):
                            for m in range(npair):
                                nc.tensor.matmul(
                                    psQ[m][:],
                                    wq_f[ft][:, m * P : (m + 1) * P],
                                    xq_t[:, ft],
                                    start=(ft == 0),
                                    stop=(ft == nf - 1),
                                )
                        for m in range(npair):
                            nc.vector.tensor_copy(QT[m][c][:], psQ[m][:])
                        # K pass
                        psK = [ps_projqk.tile([P, CH], F32, tag=f"psQK{m}", name="psK") for m in range(npair)]
                        for ft in range(nf):
                            for m in range(npair):
                                nc.tensor.matmul(
                                    psK[m][:],
                                    wk_f[ft][:, m * P : (m + 1) * P],
                                    xkv_t[:, ft],
                                    start=(ft == 0),
                                    stop=(ft == nf - 1),
                                )
                        for m in range(npair):
                            nc.vector.tensor_copy(KT[m][c][:], psK[m][:])
                        # V pass (xkv chunk tile as lhsT); one PSUM
                        # accumulation group per bank, so st is outer
                        for st in range(4):
                            psV = ps_projv.tile([P, CH], F32, tag="psV", name="psV")
                            for ft in range(nf):
                                nc.tensor.matmul(
                                    psV[:, 0:hd],
                                    xkv_t[:, ft, st * P : (st + 1) * P],
                                    wv_sb[:, ft, :],
                                    start=(ft == 0),
                                    stop=(ft == nf - 1),
                                )
                            kt = c * 4 + st
                            nc.vector.tensor_copy(
                                V[kt][:, :, 0:DH],
                                psV[:, 0:hd].rearrange("p (h d) -> p h d", h=hpc),
                            )
                        # blocks 0/1's scores/exp for the k-tiles this
                        # projection chunk just enabled run on the
                        # otherwise-idle ACT engine
                        for kt in range(4 * c, 4 * c + 4):
                            emit_score_exp(0, kt)

                # deferred work queue: sub-microsecond PE units injected into
                # later kt iterations so the ACT engine stays saturated
                pending = []

                def queue_normalize(p_, c, psY):
                    def emit(h01, psY=psY):
                        den_r = npool.tile([1, CH], F32R, tag="den", name="den_r")
                        nc.vector.tensor_copy(den_r[:], psY[h01][DH : DH + 1, :])
                        ps_bc = ps_att.tile([DH, CH], F32, tag="ps_s", name="ps_bc")
                        nc.tensor.matmul(ps_bc[:], ones_sb[0:1, 0:DH], den_r[:])
                        inv_sb = npool.tile([DH, CH], F32, tag="inv", name="inv_sb")
                        nc.vector.reciprocal_approx_fast(out=inv_sb[:], in_=ps_bc[:])
                        nc.vector.tensor_tensor(
                            YT[p_][c][h01 * DH : (h01 + 1) * DH, :],
                            psY[h01][0:DH, :],
                            inv_sb[:],
                            mybir.AluOpType.mult,
                        )

                    pending.append(lambda: emit(0))
                    pending.append(lambda: emit(1))

                def queue_outproj(c):
                    for st in range(4):
                        qt = c * 4 + st
                        carrier = {}

                        def emit_half(j, st=st, c=c, carrier=carrier):
                            if j == 0:
                                carrier["out_sb"] = opool.tile([P, o], F32, tag="out_sb", name="out_sb")
                            ps_o = ps_att.tile([P, CH], F32, tag="ps_s", name="ps_o")
                            for m in range(hd // P):
                                nc.tensor.matmul(
                                    ps_o[:],
                                    YT[m][c][:, st * P : (st + 1) * P],
                                    wo_sb[:, m, j * CH : (j + 1) * CH],
                                    start=(m == 0),
                                    stop=(m == hd // P - 1),
                                )
                            nc.vector.tensor_copy(
                                carrier["out_sb"][:, j * CH : (j + 1) * CH], ps_o[:]
                            )

                        def emit_dma(qt=qt, carrier=carrier):
                            nc.sync.dma_start(
                                out.ap()[qt * P : (qt + 1) * P, :], carrier["out_sb"][:]
                            )

                        pending.append(lambda f_=emit_half: f_(0))
                        pending.append(lambda f_=emit_half: f_(1))
                        pending.append(emit_dma)

                # block-level pipeline: during block bi's y-phase, block
                # bi+1's scores/exp stream on ACT (block0's ran under the
                # projections), so y-matmuls never wait on in-flight exps
                with tc.tile_pool(name="ps_y", bufs=2, space="PSUM") as ps_ypool:
                    for bi, (c, p_) in enumerate(blocks):
                        hA, hB = 2 * p_, 2 * p_ + 1
                        psY = [
                            ps_ypool.tile([DH + 1, CH], F32, tag=f"psY{h}", name=f"psY{h}")
                            for h in (0, 1)
                        ]
                        for kt in range(nkt):
                            # y first: frees the same-parity pt slot that
                            # block bi+2's exp will overwrite
                            pt = PT.pop((bi, kt))
                            nc.tensor.matmul(
                                psY[0][:],
                                V[kt][:, hA, :],
                                pt[:, 0:CH],
                                start=(kt == 0),
                                stop=(kt == nkt - 1),
                            )
                            nc.tensor.matmul(
                                psY[1][:],
                                V[kt][:, hB, :],
                                pt[:, CH : 2 * CH],
                                start=(kt == 0),
                                stop=(kt == nkt - 1),
                            )
                            if pending and (
                                kt % 2 == 1
                                or len(pending) > 6
                                or bi >= len(blocks) - 2
                            ):
                                pending.pop(0)()
                            if bi + 1 < len(blocks) and (bi + 1, kt) not in PT:
                                emit_score_exp(bi + 1, kt)
                        queue_normalize(p_, c, psY)
                        if p_ == npair - 1:
                            queue_outproj(c)
                    while pending:
                        pending.pop(0)()

    nc.compile()
    return nc


def make_in_maps(inputs_q, inputs_kv, wq, wk, wv, wo):
    """Shard full inputs into 8 per-core input dicts (host-side)."""
    in_maps = []
    scale = 1.0 / np.sqrt(DH)
    for core in range(NCORES):
        b = core // (NCORES // B)
        hg = core % (NCORES // B)
        hs = slice(hg * HPC, (hg + 1) * HPC)
        in_maps.append(
            {
                "xqT": np.ascontiguousarray(inputs_q[b].T).astype(np.float16),
                "xkvT": np.ascontiguousarray(inputs_kv[b].T).astype(np.float16),
                "wq": np.ascontiguousarray(
                    (wq[:, hs, :] * scale).reshape(F, HPC * DH)
                ).astype(np.float16),
                "wk": np.ascontiguousarray(wk[:, hs, :].reshape(F, HPC * DH)).astype(
                    np.float16
                ),
                "wv": np.ascontiguousarray(wv[:, hs, :].reshape(F, HPC * DH)).astype(
                    np.float16
                ),
                "wo": np.ascontiguousarray(wo[hs].reshape(HPC * DH, O)).astype(
                    np.float16
                ),
            }
        )
    return in_maps


_CACHE = {}


def _get_program():
    if "nc" not in _CACHE:
        _CACHE["nc"] = build_program()
    return _CACHE["nc"]


def run_sharded(inputs_q, inputs_kv, wq, wk, wv, wo, bo, **spmd_kwargs):
    """Build in_maps, run on 8 cores, reduce partials. Returns (out, results)."""
    nc = _get_program()
    in_maps = make_in_maps(inputs_q, inputs_kv, wq, wk, wv, wo)
    res = run_bass_kernel_spmd(nc, in_maps, core_ids=list(range(NCORES)), **spmd_kwargs)
    gpb = NCORES // B  # head-group cores per batch element
    out = np.zeros((B, S, O), dtype=np.float32)
    for core in range(NCORES):
        out[core // gpb] += res.results[core]["out"]
    out += np.asarray(bo, dtype=np.float32)
    return out, res


def kernel(inputs_q, inputs_kv, wq, wk, wv, wo, bo):
    out, _ = run_sharded(
        np.asarray(inputs_q),
        np.asarray(inputs_kv),
        np.asarray(wq),
        np.asarray(wk),
        np.asarray(wv),
        np.asarray(wo),
        np.asarray(bo),
    )
    return out


# revision 24
# speedup vs baseline: 1.0695x; 1.0274x over previous
"""Multi-head dot-product attention (B=2, S=2048, F=1024, H=16, DH=64, O=1024)
as a Bass/Tile kernel on 8 Trainium2 NeuronCores.

Sharding: data-parallel over B (2) x tensor-parallel over H (4 groups of 4
heads) = 8 cores. Each core computes q/k/v projections for its 4 heads,
softmax attention, and a partial output projection; the host sums the 4
partial outputs per batch element and adds the bias.

Device layouts (per core):
  xqT, xkvT  [F, S]  fp16   host-pre-transposed activations
  wq, wk, wv [F, 4*DH] fp16 weight shards (wq pre-scaled by 1/sqrt(DH))
  wo         [4*DH, O] fp16
  out        [S, O]  fp32   partial output

Attention works in transposed-score space: sT[k, q] = KT_slice.T @ QT (two
heads packed into PE row-groups 0-63 / 64-127), one exp on ACT covers both
heads, then y'T = V'.T @ PT where V' carries a ones column so row 64 of y'T
accumulates the softmax denominator (scores are O(1), so max-subtraction is
unnecessary). The denominator row is broadcast across partitions with a K=1
fp32r ones-matmul, reciprocated on DVE, and multiplied into fp16 yT tiles
used as lhsT of the output projection.

The emission is software-pipelined: scores run one k-tile ahead of the
exp-dependent y matmuls, the next block's first scores are issued before the
current block's normalization, and normalization + output-projection work is
spread through the following block's kt loop in sub-microsecond units so the
ACT engine (the throughput floor) never stalls.
"""

import numpy as np

import concourse.bass as bass
import concourse.mybir as mybir
import concourse.tile as tile
from concourse import bacc
from concourse.bass_utils import run_bass_kernel_spmd

F32 = mybir.dt.float32
F32R = mybir.dt.float32r
F16 = mybir.dt.float16
AF = mybir.ActivationFunctionType

B, S, F, H, DH, O = 2, 2048, 1024, 16, 64, 1024
NCORES = 8
HPC = 4  # heads per core
CH = 512  # q-chunk width
P = 128


def build_program(s=S, f=F, o=O, hpc=HPC):
    npair = hpc // 2
    nch = s // CH  # q chunks
    nkt = s // P  # k tiles
    nf = f // P  # contraction tiles for projections
    hd = hpc * DH  # stacked head dims per core (256)

    nc = bacc.Bacc("TRN2", target_bir_lowering=False, debug=False, num_devices=NCORES)

    xqT = nc.dram_tensor("xqT", [f, s], F16, kind="ExternalInput")
    xkvT = nc.dram_tensor("xkvT", [f, s], F16, kind="ExternalInput")
    wq = nc.dram_tensor("wq", [f, hd], F16, kind="ExternalInput")
    wk = nc.dram_tensor("wk", [f, hd], F16, kind="ExternalInput")
    wv = nc.dram_tensor("wv", [f, hd], F16, kind="ExternalInput")
    wo = nc.dram_tensor("wo", [hd, o], F16, kind="ExternalInput")
    out = nc.dram_tensor("out", [s, o], F32, kind="ExternalOutput")

    xqT_t = xqT.ap().rearrange("(t p) n -> p t n", p=P)  # [128, nf, s]
    xkvT_t = xkvT.ap().rearrange("(t p) n -> p t n", p=P)
    wq_t = wq.ap().rearrange("(t p) n -> p t n", p=P)  # [128, nf, hd]
    wk_t = wk.ap().rearrange("(t p) n -> p t n", p=P)
    wv_t = wv.ap().rearrange("(t p) n -> p t n", p=P)
    wo_t = wo.ap().rearrange("(t p) n -> p t n", p=P)  # [128, hd//128, o]

    with tile.TileContext(nc) as tc:
        with (
            tc.tile_pool(name="weights", bufs=1) as wpool,
            tc.tile_pool(name="xin", bufs=2) as xpool,
            tc.tile_pool(name="qkv", bufs=1) as qkvpool,
            tc.tile_pool(name="pt", bufs=1) as ptpool,
            tc.tile_pool(name="norm", bufs=4) as npool,
            tc.tile_pool(name="outsb", bufs=2) as opool,
        ):
            # ---- weights + constants -------------------------------------
            # per-f-tile wq/wk tiles so the first matmuls only wait on a
            # small DMA; weight DMAs ride the ACT HWDGE ring so they don't
            # head-of-line block the x stream on the SP ring
            wq_f = [wpool.tile([P, hd], F16, tag=f"wq{t}", name=f"wq{t}") for t in range(nf)]
            wk_f = [wpool.tile([P, hd], F16, tag=f"wk{t}", name=f"wk{t}") for t in range(nf)]
            wv_sb = wpool.tile([P, nf, hd], F16, tag="wv")
            wo_sb = wpool.tile([P, hd // P, o], F16, tag="wo")
            for ft in range(nf):
                nc.scalar.dma_start(wq_f[ft][:], wq_t[:, ft])
                nc.scalar.dma_start(wk_f[ft][:], wk_t[:, ft])
            nc.scalar.dma_start(wv_sb[:], wv_t)
            nc.scalar.dma_start(wo_sb[:], wo_t)
            # memset can't write fp16/fp32r; memset fp32 scratch, cast-copy
            ones_f32 = wpool.tile([P, P], F32, tag="ones_f32")
            nc.vector.memset(ones_f32[:], 1.0)
            ones_sb = wpool.tile([1, P], F16, tag="ones")
            nc.vector.tensor_copy(ones_sb[:], ones_f32[0:1, :])

            # ---- storage -------------------------------------------------
            QT = [
                [qkvpool.tile([P, CH], F16, tag=f"QT{p_}_{c}", name=f"QT{p_}_{c}") for c in range(nch)]
                for p_ in range(npair)
            ]
            KT = [
                [qkvpool.tile([P, CH], F16, tag=f"KT{p_}_{c}", name=f"KT{p_}_{c}") for c in range(nch)]
                for p_ in range(npair)
            ]
            # V': per k-tile [128, hpc, DH+1]; last column is ones
            V = [qkvpool.tile([P, hpc, DH + 1], F16, tag=f"V{kt}", name=f"V{kt}") for kt in range(nkt)]
            YT = [
                [qkvpool.tile([P, CH], F16, tag=f"YT{p_}_{c}", name=f"YT{p_}_{c}") for c in range(nch)]
                for p_ in range(npair)
            ]
            for kt in range(nkt):
                nc.vector.tensor_copy(V[kt][:, :, DH], ones_f32[:, 0:hpc])

            # ps_s (scores / broadcast / out-proj PSUM) lives for the whole
            # kernel: 4 banks. Projection-phase pools add 4 more (within the
            # 8-bank budget); after they close, the psY pool takes 4.
            with tc.tile_pool(name="ps_att", bufs=2, space="PSUM") as ps_att:
                blocks = [(c, p_) for c in range(nch) for p_ in range(npair)]

                def emit_scores(p_, c, kt):
                    ps_s = ps_att.tile([P, 2 * CH], F32, tag="ps_s", name="ps_s")
                    nc.tensor.matmul(
                        ps_s[:, 0:CH],
                        KT[p_][kt // 4][0:DH, (kt % 4) * P : (kt % 4 + 1) * P],
                        QT[p_][c][0:DH, :],
                        tile_position=(0, 0),
                    )
                    nc.tensor.matmul(
                        ps_s[:, CH : 2 * CH],
                        KT[p_][kt // 4][DH : 2 * DH, (kt % 4) * P : (kt % 4 + 1) * P],
                        QT[p_][c][DH : 2 * DH, :],
                        tile_position=(DH, 0),
                    )
                    return ps_s

                # saved exp(scores) tiles, written one block ahead of their
                # y-matmuls; (block parity, kt) keys the SBUF slot
                PT = {}

                def emit_score_exp(bi, kt):
                    c, p_ = blocks[bi]
                    ps_s = emit_scores(p_, c, kt)
                    pt = ptpool.tile(
                        [P, 2 * CH], F16, tag=f"pt{bi % 2}_{kt}", name=f"pt{bi % 2}_{kt}"
                    )
                    nc.scalar.activation(pt[:], ps_s[:], AF.Exp)
                    PT[(bi, kt)] = pt

                # ---- projections (+ block0 scores/exp hidden under them) --
                with (
                    tc.tile_pool(name="ps_projqk", bufs=1, space="PSUM") as ps_projqk,
                    tc.tile_pool(name="ps_projv", bufs=2, space="PSUM") as ps_projv,
                ):
                    # PE warm-up: dummy matmuls with no DMA dependency keep
                    # the PE busy through the HAM activity window so the real
                    # projection matmuls start at 2.4GHz instead of 1.2GHz
                    for wu in range(12):
                        ps_wu = ps_projv.tile([P, P], F32, tag="psV", name="ps_wu")
                        nc.tensor.matmul(ps_wu[:], ones_sb[:], ones_sb[:])
                    for c in range(nch):
                        # one 1MB DMA per chunk per stream (SP-side dispatch
                        # for [128,512] slices costs ~0.7us each)
                        xq_t = xpool.tile([P, nf, CH], F16, tag="xq", name="xq_t")
                        nc.sync.dma_start(xq_t[:], xqT_t[:, :, c * CH : (c + 1) * CH])
                        xkv_t = xpool.tile([P, nf, CH], F16, tag="xkv", name="xkv_t")
                        nc.sync.dma_start(xkv_t[:], xkvT_t[:, :, c * CH : (c + 1) * CH])
                        # Q pass (K pass reuses the same PSUM tags)
                        psQ = [ps_projqk.tile([P, CH], F32, tag=f"psQK{m}", name="psQ") for m in range(npair)]
                        for ft in range(nf):
                            for m in range(npair):
                                nc.tensor.matmul(
                                    psQ[m][:],
                                    wq_f[ft][:, m * P : (m + 1) * P],
                                    xq_t[:, ft],
                                    start=(ft == 0),
                                    stop=(ft == nf - 1),
                                )
                        for m in range(npair):
                            nc.vector.tensor_copy(QT[m][c][:], psQ[m][:])
                        # K pass
                        psK = [ps_projqk.tile([P, CH], F32, tag=f"psQK{m}", name="psK") for m in range(npair)]
                        for ft in range(nf):
                            for m in range(npair):
                                nc.tensor.matmul(
                                    psK[m][:],
                                    wk_f[ft][:, m * P : (m + 1) * P],
                                    xkv_t[:, ft],
                                    start=(ft == 0),
                                    stop=(ft == nf - 1),
                                )
                        for m in range(npair):
                            nc.vector.tensor_copy(KT[m][c][:], psK[m][:])
                        # V pass (xkv chunk tile as lhsT); one PSUM
                        # accumulation group per bank, so st is outer
                        for st in range(4):
                            psV = ps_projv.tile([P, CH], F32, tag="psV", name="psV")
                            for ft in range(nf):
                                nc.tensor.matmul(
                                    psV[:, 0:hd],
                                    xkv_t[:, ft, st * P : (st + 1) * P],
                                    wv_sb[:, ft, :],
                                    start=(ft == 0),
                                    stop=(ft == nf - 1),
                                )
                            kt = c * 4 + st
                            nc.vector.tensor_copy(
                                V[kt][:, :, 0:DH],
                                psV[:, 0:hd].rearrange("p (h d) -> p h d", h=hpc),
                            )
                        # blocks 0/1's scores/exp for the k-tiles this
                        # projection chunk just enabled run on the
                        # otherwise-idle ACT engine
                        for kt in range(4 * c, 4 * c + 4):
                            emit_score_exp(0, kt)

                # deferred work queue: sub-microsecond PE units injected into
                # later kt iterations so the ACT engine stays saturated
                pending = []

                def queue_normalize(p_, c, psY):
                    def emit(h01, psY=psY):
                        den_r = npool.tile([1, CH], F16, tag="den", name="den_r")
                        nc.vector.tensor_copy(den_r[:], psY[h01][DH : DH + 1, :])
                        ps_bc = ps_att.tile([DH, CH], F32, tag="ps_s", name="ps_bc")
                        nc.tensor.matmul(ps_bc[:], ones_sb[0:1, 0:DH], den_r[:])
                        inv_sb = npool.tile([DH, CH], F32, tag="inv", name="inv_sb")
                        nc.vector.reciprocal_approx_fast(out=inv_sb[:], in_=ps_bc[:])
                        nc.vector.tensor_tensor(
                            YT[p_][c][h01 * DH : (h01 + 1) * DH, :],
                            psY[h01][0:DH, :],
                            inv_sb[:],
                            mybir.AluOpType.mult,
                        )

                    pending.append(lambda: emit(0))
                    pending.append(lambda: emit(1))

                def queue_outproj(c):
                    for st in range(4):
                        qt = c * 4 + st
                        carrier = {}

                        def emit_half(j, st=st, c=c, carrier=carrier):
                            if j == 0:
                                carrier["out_sb"] = opool.tile([P, o], F32, tag="out_sb", name="out_sb")
                            ps_o = ps_att.tile([P, CH], F32, tag="ps_s", name="ps_o")
                            for m in range(hd // P):
                                nc.tensor.matmul(
                                    ps_o[:],
                                    YT[m][c][:, st * P : (st + 1) * P],
                                    wo_sb[:, m, j * CH : (j + 1) * CH],
                                    start=(m == 0),
                                    stop=(m == hd // P - 1),
                                )
                            nc.vector.tensor_copy(
                                carrier["out_sb"][:, j * CH : (j + 1) * CH], ps_o[:]
                            )

                        def emit_dma(qt=qt, carrier=carrier):
                            nc.sync.dma_start(
                                out.ap()[qt * P : (qt + 1) * P, :], carrier["out_sb"][:]
                            )

                        pending.append(lambda f_=emit_half: f_(0))
                        pending.append(lambda f_=emit_half: f_(1))
                        pending.append(emit_dma)

                # block-level pipeline: during block bi's y-phase, block
                # bi+1's scores/exp stream on ACT (block0's ran under the
                # projections), so y-matmuls never wait on in-flight exps
                with tc.tile_pool(name="ps_y", bufs=2, space="PSUM") as ps_ypool:
                    for bi, (c, p_) in enumerate(blocks):
                        hA, hB = 2 * p_, 2 * p_ + 1
                        psY = [
                            ps_ypool.tile([DH + 1, CH], F32, tag=f"psY{h}", name=f"psY{h}")
                            for h in (0, 1)
                        ]
                        for kt in range(nkt):
                            # y first: frees the same-parity pt slot that
                            # block bi+2's exp will overwrite
                            pt = PT.pop((bi, kt))
                            nc.tensor.matmul(
                                psY[0][:],
                                V[kt][:, hA, :],
                                pt[:, 0:CH],
                                start=(kt == 0),
                                stop=(kt == nkt - 1),
                            )
                            nc.tensor.matmul(
                                psY[1][:],
                                V[kt][:, hB, :],
                                pt[:, CH : 2 * CH],
                                start=(kt == 0),
                                stop=(kt == nkt - 1),
                            )
                            if pending and (
                                kt % 2 == 1
                                or len(pending) > 6
                                or bi >= len(blocks) - 2
                            ):
                                pending.pop(0)()
                            if bi + 1 < len(blocks) and (bi + 1, kt) not in PT:
                                emit_score_exp(bi + 1, kt)
                        queue_normalize(p_, c, psY)
                        if p_ == npair - 1:
                            queue_outproj(c)
                    while pending:
                        pending.pop(0)()

    nc.compile()
    return nc


def make_in_maps(inputs_q, inputs_kv, wq, wk, wv, wo):
    """Shard full inputs into 8 per-core input dicts (host-side)."""
    in_maps = []
    scale = 1.0 / np.sqrt(DH)
    for core in range(NCORES):
        b = core // (NCORES // B)
        hg = core % (NCORES // B)
        hs = slice(hg * HPC, (hg + 1) * HPC)
        in_maps.append(
            {
                "xqT": np.ascontiguousarray(inputs_q[b].T).astype(np.float16),
                "xkvT": np.ascontiguousarray(inputs_kv[b].T).astype(np.float16),
                "wq": np.ascontiguousarray(
                    (wq[:, hs, :] * scale).reshape(F, HPC * DH)
                ).astype(np.float16),
                "wk": np.ascontiguousarray(wk[:, hs, :].reshape(F, HPC * DH)).astype(
                    np.float16
                ),
                "wv": np.ascontiguousarray(wv[:, hs, :].reshape(F, HPC * DH)).astype(
                    np.float16
                ),
                "wo": np.ascontiguousarray(wo[hs].reshape(HPC * DH, O)).astype(
                    np.float16
                ),
            }
        )
    return in_maps


_CACHE = {}


def _get_program():
    if "nc" not in _CACHE:
        _CACHE["nc"] = build_program()
    return _CACHE["nc"]


def run_sharded(inputs_q, inputs_kv, wq, wk, wv, wo, bo, **spmd_kwargs):
    """Build in_maps, run on 8 cores, reduce partials. Returns (out, results)."""
    nc = _get_program()
    in_maps = make_in_maps(inputs_q, inputs_kv, wq, wk, wv, wo)
    res = run_bass_kernel_spmd(nc, in_maps, core_ids=list(range(NCORES)), **spmd_kwargs)
    gpb = NCORES // B  # head-group cores per batch element
    out = np.zeros((B, S, O), dtype=np.float32)
    for core in range(NCORES):
        out[core // gpb] += res.results[core]["out"]
    out += np.asarray(bo, dtype=np.float32)
    return out, res


def kernel(inputs_q, inputs_kv, wq, wk, wv, wo, bo):
    out, _ = run_sharded(
        np.asarray(inputs_q),
        np.asarray(inputs_kv),
        np.asarray(wq),
        np.asarray(wk),
        np.asarray(wv),
        np.asarray(wo),
        np.asarray(bo),
    )
    return out
